# revision 47
# baseline (speedup 1.0000x reference)
"""Chamfer-augmented kernel for Trainium2 (8 NeuronCores, data-parallel over batch).

For each batch b and each grid sample s:
    mins[s]  = min_j ||grid_s - pred_j||
    mins2[s] = min_j ||grid_s - gt_j||
    out[b]   = mean_s |mins - mins2|

Per-core algorithm (batch b on core b):
  d^2(s,j) = x_s^2 + (q_j - 2 x_s . y_j)  with q_j = ||y_j||^2.
  The (q - 2 x.y) term is produced by a single K=18 bf16 matmul using an exact
  Karatsuba split (x = xh + xl, y' = -2y = yh + yl, q = qh + ql per coordinate):
    rows: [xh,xh,xl,xl,1,1] (x3 coords)  vs  [yh,yl,yh,yl,qh,ql] (x3 coords)
  The distance matrix is never materialized: matmuls stream [128, 1024] PSUM
  groups, whose first touch is split between the two engines that can read
  PSUM: per m-tile, 6 groups go to ScalarE (Identity convert -> fp16 SBUF with
  a fused per-partition +x^2 bias, so fp16 rounding is relative to d^2), and 2
  groups go to VectorE reduce_min directly. VectorE then folds the fp16 groups
  with a 2x-rate elementwise min tree. This balances ScalarE/VectorE at ~6us
  per m-tile each while the TensorE matmuls (~109us total) hide underneath.
"""

import os

import numpy as np

import concourse.bass as bass
import concourse.tile as tile
from concourse import bacc, mybir, bass_utils

F32 = mybir.dt.float32
BF16 = mybir.dt.bfloat16
F16 = mybir.dt.float16
AX = mybir.AxisListType
OP = mybir.AluOpType
AF = mybir.ActivationFunctionType

# Units (set, m) whose groups ALL go through the ScalarE fp16-convert path;
# the remaining units send their last group to the VectorE-direct reduce so
# both engines consume PSUM concurrently.
ALLACT_UNITS = frozenset(
    int(x) for x in os.environ.get("CH_ALLACT", "").split(",")
    if x != "")

BS = 8
S = 2048          # n_samples (grid points)
J = 8192          # n_points (preds/gts)
NT = 512          # matmul moving tile (one PSUM bank)
GT = int(os.environ.get("CH_GT", "1024"))   # reduce group (GT/512 banks)
PSB = int(os.environ.get("CH_PSB", "4"))    # psum slots
WKB = int(os.environ.get("CH_WKB", "2"))    # work pool bufs
ND = int(os.environ.get("CH_ND", "2"))      # direct groups per mixed unit
NM = S // 128     # 16 m-tiles
NG = J // GT      # groups per m-tile
PACK = 8          # prep packing: [3*PACK, J/PACK]
JP = J // PACK


def _build_prep(nc, tc, sb, pts_dram, name):
    """Load one point set (packed [24, J/8] f32) and build the K=18 bf16 rhs.

    Returns the [18, J] bf16 rhs tile (rows: yh,yl,yh,yl,qh,ql per coord with
    y = -2*p and q = p^2 per coordinate).
    """
    Y = sb.tile([3 * PACK, JP], F32, tag=f"y_{name}")
    nc.sync.dma_start(Y[:], pts_dram)
    SQ = sb.tile([3 * PACK, JP], F32, tag=f"sq_{name}")
    nc.vector.tensor_tensor(SQ[:], Y[:], Y[:], op=OP.mult)
    YH = sb.tile([3 * PACK, JP], BF16, tag=f"yh_{name}")
    nc.scalar.activation(YH[:], Y[:], AF.Copy, scale=-2.0)
    YL = sb.tile([3 * PACK, JP], BF16, tag=f"yl_{name}")
    # yl = (-2*y) - yh, rounded to bf16
    nc.vector.scalar_tensor_tensor(YL[:], Y[:], -2.0, YH[:], op0=OP.mult, op1=OP.subtract)
    QH = sb.tile([3 * PACK, JP], BF16, tag=f"qh_{name}")
    nc.scalar.activation(QH[:], SQ[:], AF.Copy)
    QL = sb.tile([3 * PACK, JP], BF16, tag=f"ql_{name}")
    nc.vector.tensor_tensor(QL[:], SQ[:], QH[:], op=OP.subtract)

    RH = sb.tile([18, J], BF16, tag=f"rh_{name}")
    # unpack [3*PACK, JP] -> [3, J]; AP iteration orders match (d, chunk, j).
    for i, src in enumerate((YH, YL, YH, YL, QH, QL)):
        nc.sync.dma_start(RH[3 * i:3 * i + 3, :], src[:])
    return RH


def _minloop(nc, wk, ps, ps2, LH, RH, MINS, X2, gmtag, unit0):
    """Per m-tile: 4 ScalarE groups of 1536 (3-bank psum, wider ops amortize
    the SBUF-write bubble) + 2 VectorE-direct groups of 1024 (2-bank psum)."""
    for m in range(NM):
        CC = wk.tile([128, 6144], F16, tag="cc")
        for ga in range(4):
            PA = ps.tile([128, 1536], F32, tag="pga")
            for t in range(3):
                j0 = ga * 1536 + t * 512
                nc.tensor.matmul(
                    PA[:, t * 512:(t + 1) * 512],
                    LH[:, m * 128:(m + 1) * 128],
                    RH[:, j0:j0 + 512],
                    start=True, stop=True,
                )
            nc.scalar.activation(CC[:, ga * 1536:(ga + 1) * 1536], PA[:],
                                 AF.Identity, bias=X2[:, m:m + 1])
        GMD = wk.tile([128, 2], F32, tag=gmtag)
        for gd in range(2):
            PD = ps2.tile([128, 1024], F32, tag="pgd")
            for t in range(2):
                j0 = 6144 + gd * 1024 + t * 512
                nc.tensor.matmul(
                    PD[:, t * 512:(t + 1) * 512],
                    LH[:, m * 128:(m + 1) * 128],
                    RH[:, j0:j0 + 512],
                    start=True, stop=True,
                )
            nc.vector.tensor_reduce(GMD[:, gd:gd + 1], PD[:], axis=AX.X, op=OP.min)
        GMF = wk.tile([128, 1], F32, tag=gmtag + "f")
        nc.vector.tensor_reduce(GMF[:], GMD[:], axis=AX.X, op=OP.min)
        T = CC
        W = 6144
        lvl = 0
        # halving (2x-rate tt) beats the 1x final reduce while W > 232
        while W > 232:
            W2 = W // 2
            TN = wk.tile([128, W2], F16, tag=f"tr{lvl}")
            nc.vector.tensor_tensor(TN[:], T[:, :W2], T[:, W2:2 * W2], op=OP.min)
            T, W, lvl = TN, W2, lvl + 1
        D2C = wk.tile([128, 1], F32, tag="d2c")
        nc.vector.tensor_reduce(D2C[:], T[:, :W], axis=AX.X, op=OP.min)
        nc.vector.scalar_tensor_tensor(MINS[:, m:m + 1], GMF[:],
                                       X2[:, m:m + 1], D2C[:],
                                       op0=OP.add, op1=OP.min)


def _build_module():
    nc = bacc.Bacc("TRN2", target_bir_lowering=False, debug=False, num_devices=BS)
    grid_t = nc.dram_tensor("grid_t", [3, S], F32, kind="ExternalInput").ap()
    grid_n = nc.dram_tensor("grid_n", [128, 3 * NM], F32, kind="ExternalInput").ap()
    preds_p = nc.dram_tensor("preds_p", [3 * PACK, JP], F32, kind="ExternalInput").ap()
    gts_p = nc.dram_tensor("gts_p", [3 * PACK, JP], F32, kind="ExternalInput").ap()
    out_d = nc.dram_tensor("out", [1, 1], F32, kind="ExternalOutput").ap()

    with tile.TileContext(nc) as tc:
        with tc.tile_pool(name="sb", bufs=1) as sb, \
             tc.tile_pool(name="wk", bufs=WKB) as wk, \
             tc.tile_pool(name="ps", bufs=2, space="PSUM") as ps, \
             tc.tile_pool(name="ps2", bufs=1, space="PSUM") as ps2:
            # ---- lhsT [18, S]: [xh,xh,xl,xl,1,1] per coordinate ----
            GT_ = sb.tile([3, S], F32, tag="gridt")
            nc.sync.dma_start(GT_[:], grid_t)
            XH = sb.tile([3, S], BF16, tag="xh")
            nc.scalar.activation(XH[:], GT_[:], AF.Copy)
            XL = sb.tile([3, S], BF16, tag="xl")
            nc.vector.tensor_tensor(XL[:], GT_[:], XH[:], op=OP.subtract)
            ONES = sb.tile([3, S], BF16, tag="ones")
            nc.vector.memset(ONES[:], 1.0)
            LH = sb.tile([18, S], BF16, tag="lh")
            for i, src in enumerate((XH, XH, XL, XL, ONES, ONES)):
                nc.sync.dma_start(LH[3 * i:3 * i + 3, :], src[:])

            # ---- x^2 per (p, m) from the untransposed grid copy ----
            GN = sb.tile([128, 3 * NM], F32, tag="gn")
            nc.sync.dma_start(GN[:], grid_n)
            GN2 = sb.tile([128, 3 * NM], F32, tag="gn2")
            nc.vector.tensor_tensor(GN2[:], GN[:], GN[:], op=OP.mult)
            X2 = sb.tile([128, NM], F32, tag="x2")
            g3 = GN2[:].rearrange("p (m d) -> p m d", d=3)
            nc.vector.tensor_tensor(X2[:], g3[:, :, 0], g3[:, :, 1], op=OP.add)
            nc.vector.tensor_tensor(X2[:], X2[:], g3[:, :, 2], op=OP.add)

            # ---- per-set rhs prep + min loop + distance finalization ----
            def _distances(MINS, tag):
                # d = sqrt(max(d^2, eps)) with one Newton refinement step
                D2 = sb.tile([128, NM], F32, tag=f"d2{tag}")
                nc.vector.tensor_scalar_max(D2[:], MINS[:], 1e-12)
                D0 = sb.tile([128, NM], F32, tag=f"d0{tag}")
                nc.scalar.activation(D0[:], D2[:], AF.Sqrt)
                R = sb.tile([128, NM], F32, tag=f"r{tag}")
                nc.vector.reciprocal(R[:], D0[:])
                # d1 = 0.5*(d0 + d2/d0)
                D1 = sb.tile([128, NM], F32, tag=f"d1{tag}")
                nc.vector.tensor_tensor(D1[:], D2[:], R[:], op=OP.mult)
                nc.vector.tensor_tensor(D1[:], D1[:], D0[:], op=OP.add)
                nc.vector.tensor_scalar_mul(D1[:], D1[:], 0.5)
                return D1

            RHP = _build_prep(nc, tc, sb, preds_p, "p")
            RHG = _build_prep(nc, tc, sb, gts_p, "g")

            MINS_P = sb.tile([128, NM], F32, tag="minsp")
            MINS_G = sb.tile([128, NM], F32, tag="minsg")
            _minloop(nc, wk, ps, ps2, LH, RHP, MINS_P, X2, "gmp", 0)
            _minloop(nc, wk, ps, ps2, LH, RHG, MINS_G, X2, "gmg", NM)
            DS = [_distances(MINS_P, "dp"), _distances(MINS_G, "dg")]

            # ---- mean_s |dp - dg| ----
            DIFF = sb.tile([128, NM], F32, tag="diff")
            nc.vector.tensor_tensor(DIFF[:], DS[0][:], DS[1][:], op=OP.subtract)
            SROW = sb.tile([128, 1], F32, tag="srow")
            nc.vector.tensor_reduce(SROW[:], DIFF[:], axis=AX.X, op=OP.add,
                                    apply_absolute_value=True)
            ONE32 = sb.tile([128, 1], F32, tag="one32")
            nc.vector.memset(ONE32[:], 1.0)
            # reuse a psum "pg" slot for the final [1,1] accumulation
            PGX = ps.tile([128, 1536], F32, tag="pga")
            TOT = PGX[0:1, 0:1]
            nc.tensor.matmul(TOT, ONE32[:], SROW[:], start=True, stop=True)
            OUT = sb.tile([1, 1], F32, tag="outsb")
            nc.scalar.activation(OUT[:], TOT, AF.Copy, scale=1.0 / float(S))
            nc.sync.dma_start(out_d, OUT[:])
    nc.compile()
    return nc


_NC = None


def _get_nc():
    global _NC
    if _NC is None:
        _NC = _build_module()
    return _NC


def _in_maps(gts, preds, grid_points):
    maps = []
    for b in range(BS):
        g = np.ascontiguousarray(grid_points[b], np.float32)
        maps.append({
            "grid_t": np.ascontiguousarray(g.T),
            "grid_n": np.ascontiguousarray(
                g.reshape(NM, 128, 3).transpose(1, 0, 2).reshape(128, 3 * NM)),
            "preds_p": np.ascontiguousarray(preds[b], np.float32).T.reshape(3 * PACK, JP).copy(),
            "gts_p": np.ascontiguousarray(gts[b], np.float32).T.reshape(3 * PACK, JP).copy(),
        })
    return maps


def kernel(gts, preds, grid_points, _trace=False, _trace_kwargs=None):
    nc = _get_nc()
    res = bass_utils.run_bass_kernel_spmd(
        nc, _in_maps(gts, preds, grid_points), core_ids=list(range(BS)),
        trace=_trace, **(_trace_kwargs or {}))
    out = np.array([res.results[b]["out"][0, 0] for b in range(BS)], np.float32)
    if _trace:
        return out, res
    return out
